# revision 31
# baseline (speedup 1.0000x reference)
"""Trainium2 Bass kernel for EmformerAttention (talking-heads attention).

Sharding: 8 cores; core i handles batch b = i//4, q-rows [(i%4)*580, +580).
Device pipeline per core (all fp16 matmul operands, fp32 PSUM):
  Q/K/V projections -> per-head scores (2-head row-packed tile_position) ->
  evict fp16 -> SBUF->SBUF DMA interleave to [(16q,8h), k] tiles ->
  pre-softmax talking-heads mix as block-diagonal matmul + mask rows via
  replication-selector matmul (same PSUM accumulation group) -> ACT exp with
  accum_out row-sums -> post-softmax mix as block-diagonal matmul with 1/Z
  folded into the stationary rows -> evict fp16 -> DMA-transpose to
  [k-part, (16q,8n)] -> @V with strided-free rhs -> Wout projection.
"""

import numpy as np

P = 128
B, H, D, DK = 2, 8, 512, 64
C, R, S, M = 2048, 256, 16, 15
Q, K = R + C + S, M + R + C          # 2320, 2319
QSH = 2320 // 4                      # 580 real q rows per core
QP = 640                             # padded per-core q rows (5 qblocks)
KP = 2432                            # padded keys = 19*128
NQB = QP // P                        # 5
NKT = KP // P                        # 19
CHUNKS = [(0, 1024), (1024, 1024), (2048, 271)]   # premix/exp k-chunks (real K only)
EXP_BIAS = -4.0
MASK_VAL = -30000.0
SCALE = DK ** -0.5

_BUILT = {}


def _build_program(debug=False, trace_sim=False):
    import concourse.bacc as bacc
    import concourse.mybir as mybir
    import concourse.tile as tile

    dt = mybir.dt
    AF = mybir.ActivationFunctionType
    AL = mybir.AluOpType

    nc = bacc.Bacc("TRN2", target_bir_lowering=False, debug=False)

    qinT = nc.dram_tensor("qinT", [D, QP], dt.float16, kind="ExternalInput")
    kvinT = nc.dram_tensor("kvinT", [D, KP], dt.float16, kind="ExternalInput")
    wq_d = nc.dram_tensor("wq", [D, D], dt.float16, kind="ExternalInput")
    wk_d = nc.dram_tensor("wk", [D, D], dt.float16, kind="ExternalInput")
    wv_d = nc.dram_tensor("wv", [D, D], dt.float16, kind="ExternalInput")
    wo_d = nc.dram_tensor("wo", [D, D], dt.float16, kind="ExternalInput")
    bq_d = nc.dram_tensor("bq", [D], dt.float32, kind="ExternalInput")     # pre-scaled by 0.125
    bk_d = nc.dram_tensor("bk", [D], dt.float32, kind="ExternalInput")
    bvt_d = nc.dram_tensor("bvt", [P, D], dt.float16, kind="ExternalInput")  # bv broadcast
    bot_d = nc.dram_tensor("bot", [P, D], dt.float32, kind="ExternalInput")  # bout broadcast
    mask_d = nc.dram_tensor("maskr", [QP, KP], dt.float16, kind="ExternalInput")
    prebd_d = nc.dram_tensor("prebd", [P, P], dt.float16, kind="ExternalInput")
    postbd_d = nc.dram_tensor("postbd", [P, P], dt.float16, kind="ExternalInput")
    rep_d = nc.dram_tensor("rep", [P, 8 * P], dt.float16, kind="ExternalInput")
    out_d = nc.dram_tensor("out", [QP, D], dt.float32, kind="ExternalOutput")
    if debug:
        dS = nc.dram_tensor("dS", [P, H, KP], dt.float16, kind="ExternalOutput")
        dsint = nc.dram_tensor("dsint", [P, KP], dt.float16, kind="ExternalOutput")
        de = nc.dram_tensor("de", [P, KP], dt.float16, kind="ExternalOutput")
        dz = nc.dram_tensor("dz", [P, 4], dt.float32, kind="ExternalOutput")
        dPT = nc.dram_tensor("dPT", [P, 8, NKT, P], dt.float16, kind="ExternalOutput")
        dattE = nc.dram_tensor("dattE", [64, H, P], dt.float16, kind="ExternalOutput")
        dqT = nc.dram_tensor("dqT", [P, 4, QP], dt.float16, kind="ExternalOutput")
        dkT = nc.dram_tensor("dkT", [P, 4, KP], dt.float16, kind="ExternalOutput")
        dV = nc.dram_tensor("dV", [P, NKT, D], dt.float16, kind="ExternalOutput")

    with tile.TileContext(nc, trace_sim=trace_sim) as tc:
        with (
            tc.tile_pool(name="const", bufs=1) as cpool,
            tc.tile_pool(name="proj", bufs=2) as jpool,
            tc.tile_pool(name="work", bufs=1) as wkpool,
            tc.tile_pool(name="sint", bufs=2) as ipool,
            tc.tile_pool(name="epool", bufs=2) as epool,
            tc.tile_pool(name="small", bufs=8) as smpool,
            tc.tile_pool(name="mask", bufs=2) as mkpool,
            tc.tile_pool(name="outp", bufs=2) as opool,
            tc.tile_pool(name="psc", bufs=2, space="PSUM") as psc,     # small [128,512]
            tc.tile_pool(name="pmix", bufs=3, space="PSUM") as pmix,   # big   [128,1024]
        ):
            # ---------------- constants / weights ----------------
            wo_sb = cpool.tile([P, 4, D], dt.float16)
            nc.sync.dma_start(wo_sb[:], wo_d.rearrange("(c p) d -> p c d", p=P))
            bq_sb = cpool.tile([P, 4], dt.float32)
            nc.sync.dma_start(bq_sb[:], bq_d.rearrange("(c p) -> p c", p=P))
            bk_sb = cpool.tile([P, 4], dt.float32)
            nc.sync.dma_start(bk_sb[:], bk_d.rearrange("(c p) -> p c", p=P))
            bvt_sb = cpool.tile([P, D], dt.float16)
            nc.sync.dma_start(bvt_sb[:], bvt_d[:])
            bot_sb = cpool.tile([P, D], dt.float32)
            nc.sync.dma_start(bot_sb[:], bot_d[:])
            prebd_sb = cpool.tile([P, P], dt.float16)
            nc.sync.dma_start(prebd_sb[:], prebd_d[:])
            postbd_sb = cpool.tile([P, P], dt.float16)
            nc.sync.dma_start(postbd_sb[:], postbd_d[:])
            rep_sb = cpool.tile([P, 8 * P], dt.float16)
            nc.sync.dma_start(rep_sb[:], rep_d[:])
            ebias_sb = cpool.tile([P, 1], dt.float32)
            nc.any.memset(ebias_sb[:], EXP_BIAS)

            # ---------------- projections ----------------
            qT = cpool.tile([P, 4, QP], dt.float16)
            kT = cpool.tile([P, 4, KP], dt.float16)
            V = cpool.tile([P, NKT, D], dt.float16)
            prep_ctx = tc.tile_pool(name="prep", bufs=1)
            prpool = prep_ctx.__enter__()
            wq_sb = prpool.tile([P, 4, D], dt.float16, tag="w")
            nc.sync.dma_start(wq_sb[:], wq_d.rearrange("(c p) d -> p c d", p=P))
            wk_sb = prpool.tile([P, 4, D], dt.float16, tag="w")
            nc.sync.dma_start(wk_sb[:], wk_d.rearrange("(c p) d -> p c d", p=P))
            wv_sb = prpool.tile([P, 4, D], dt.float16, tag="w")
            nc.sync.dma_start(wv_sb[:], wv_d.rearrange("(c p) d -> p c d", p=P))
            qin_sb = prpool.tile([P, 4, QP], dt.float16)
            nc.sync.dma_start(qin_sb[:], qinT.rearrange("(c p) q -> p c q", p=P))
            kvin_sb = prpool.tile([P, 4, KP], dt.float16)
            nc.sync.dma_start(kvin_sb[:], kvinT.rearrange("(c p) k -> p c k", p=P))
            for co in range(4):
                for n0, nw in ((0, 512), (512, 128)):
                    ps = psc.tile([P, 512], dt.float32, tag="psx")
                    for c in range(4):
                        nc.tensor.matmul(
                            ps[:, 0:nw],
                            wq_sb[:, c, co * P:(co + 1) * P],
                            qin_sb[:, c, n0:n0 + nw],
                            start=(c == 0), stop=(c == 3))
                    nc.scalar.activation(qT[:, co, n0:n0 + nw], ps[:, 0:nw],
                                         AF.Copy, scale=SCALE)
                nc.vector.tensor_scalar(qT[:, co, :], qT[:, co, :],
                                        bq_sb[:, co:co + 1], None, op0=AL.add)

            # kT [dout, k] fp16 as [128, 4, KP]
            for co in range(4):
                for n0, nw in ((0, 512), (512, 512), (1024, 512), (1536, 512), (2048, 384)):
                    ps = psc.tile([P, 512], dt.float32, tag="psx")
                    for c in range(4):
                        nc.tensor.matmul(
                            ps[:, 0:nw],
                            wk_sb[:, c, co * P:(co + 1) * P],
                            kvin_sb[:, c, n0:n0 + nw],
                            start=(c == 0), stop=(c == 3))
                    nc.scalar.activation(kT[:, co, n0:n0 + nw], ps[:, 0:nw], AF.Copy)
                nc.vector.tensor_scalar(kT[:, co, :], kT[:, co, :],
                                        bk_sb[:, co:co + 1], None, op0=AL.add)

            # V [k, dout] fp16 as [128, 19, D]
            for kt in range(NKT):
                ps = psc.tile([P, 512], dt.float32, tag="psx")
                for c in range(4):
                    nc.tensor.matmul(
                        ps[:],
                        kvin_sb[:, c, kt * P:(kt + 1) * P],
                        wv_sb[:, c, :],
                        start=(c == 0), stop=(c == 3))
                nc.scalar.activation(V[:, kt, :], ps[:], AF.Copy)
                nr = P if kt < NKT - 1 else K - 18 * P
                nc.vector.tensor_tensor(V[0:nr, kt, :], V[0:nr, kt, :],
                                        bvt_sb[0:nr, :], AL.add)
            prep_ctx.__exit__(None, None, None)
            if debug:
                nc.sync.dma_start(dqT[:], qT[:])
                nc.sync.dma_start(dkT[:], kT[:])
                nc.sync.dma_start(dV[:], V[:])

            # ---------------- main loop over qblocks ----------------
            for qb in range(NQB):
                qsl = slice(qb * P, (qb + 1) * P)
                mask_sb = mkpool.tile([P, KP], dt.float16, tag="mask")
                nc.sync.dma_start(mask_sb[:, 0:K], mask_d[qsl, 0:K])

                # scores: per head pair, K=64 contraction on row-groups 0/64
                S_sb = wkpool.tile([P, H, KP], dt.float16, tag="S")
                for hp in range(4):
                    for k0, kw in ((0, 512), (512, 512), (1024, 512), (1536, 512), (2048, 271)):
                        psAB = pmix.tile([P, 1024], dt.float32, tag="pmix")
                        psA = psAB[:, 0:512]
                        psB = psAB[:, 512:1024]
                        nc.tensor.matmul(
                            psA[:, 0:kw],
                            qT[0:64, hp, qsl], kT[0:64, hp, k0:k0 + kw],
                            start=True, stop=True, tile_position=(0, 0))
                        nc.tensor.matmul(
                            psB[:, 0:kw],
                            qT[64:128, hp, qsl], kT[64:128, hp, k0:k0 + kw],
                            start=True, stop=True, tile_position=(64, 0))
                        nc.vector.tensor_copy(S_sb[:, 2 * hp, k0:k0 + kw], psA[:, 0:kw])
                        nc.scalar.activation(S_sb[:, 2 * hp + 1, k0:k0 + kw],
                                             psB[:, 0:kw], AF.Copy)

                if debug and qb == 0:
                    nc.sync.dma_start(dS[:], S_sb[:])
                # PT holds transposed P for the whole qblock
                PT = wkpool.tile([P, 8, NKT, P], dt.float16, tag="PT")

                for t in range(8):
                    # interleave: partitions (16 q) x (8 h) -> 128
                    sint = ipool.tile([P, KP], dt.float16, tag="sint")
                    eng = nc.sync if t % 2 == 0 else nc.gpsimd
                    eng.dma_start(sint[:, 0:K], S_sb[t * 16:(t + 1) * 16, :, 0:K])

                    e_t = epool.tile([P, KP], dt.float16, tag="E")
                    nc.any.memset(e_t[:, K:KP], 0.0)
                    zp = smpool.tile([P, 4], dt.float32, tag="zp")
                    for j, (c0, cw) in enumerate(CHUNKS):
                        ps = pmix.tile([P, 1024], dt.float32, tag="pmix")
                        for s0 in range(0, cw, 512):
                            sw = min(512, cw - s0)
                            nc.tensor.matmul(
                                ps[:, s0:s0 + sw], prebd_sb[:],
                                sint[:, c0 + s0:c0 + s0 + sw],
                                start=True, stop=False)
                            nc.tensor.matmul(
                                ps[:, s0:s0 + sw],
                                rep_sb[:, t * P:(t + 1) * P],
                                mask_sb[:, c0 + s0:c0 + s0 + sw],
                                start=False, stop=True)
                        nc.scalar.activation(e_t[:, c0:c0 + cw], ps[:, 0:cw],
                                             AF.Exp, bias=ebias_sb[:],
                                             accum_out=zp[:, j:j + 1])
                    # Z and 1/Z
                    zs = smpool.tile([P, 1], dt.float32, tag="zs")
                    nc.vector.reduce_sum(zs[:], zp[:, 0:3], axis=mybir.AxisListType.X)
                    zr = smpool.tile([P, 1], dt.float32, tag="zr")
                    nc.vector.reciprocal(zr[:], zs[:])
                    # scaled post blockdiag: row (q,m) scaled by 1/Z[q,m]
                    scbd = smpool.tile([P, P], dt.float16, tag="scbd")
                    nc.vector.tensor_scalar(scbd[:], postbd_sb[:], zr[:],
                                            None, op0=AL.mult)

                    # postmix fused with transpose: P^T chunk = E_chunk.T @ scbd
                    for g0 in range(0, NKT, 4):
                        gs = min(4, NKT - g0)
                        pp = psc.tile([P, 512], dt.float32, tag="psx")
                        for j in range(gs):
                            kc = g0 + j
                            nc.tensor.matmul(
                                pp[:, j * P:(j + 1) * P],
                                e_t[:, kc * P:(kc + 1) * P], scbd[:],
                                start=True, stop=True)
                        if g0 in (0, 8, 16):
                            nc.vector.tensor_copy(
                                PT[:, t, g0:g0 + gs, :], pp[:, 0:gs * P])
                        else:
                            nc.scalar.activation(
                                PT[:, t, g0:g0 + gs, :], pp[:, 0:gs * P], AF.Copy)
                    if debug and qb == 0 and t == 0:
                        nc.sync.dma_start(dsint[:], sint[:])
                        nc.sync.dma_start(de[:], e_t[:])
                        nc.sync.dma_start(dz[:], zp[:])

                # @V: att^T[n] [64, 128q] accumulated over k-chunks
                attE = opool.tile([64, H, P], dt.float16, tag="attE")
                for n in range(H):
                    aps = psc.tile([P, 512], dt.float32, tag="psx")
                    for kt in range(NKT):
                        rhs = PT[:, :, kt, n:n + 121:8]
                        nc.tensor.matmul(
                            aps[0:64, 0:P],
                            V[:, kt, n * 64:(n + 1) * 64],
                            rhs,
                            start=(kt == 0), stop=(kt == NKT - 1))
                    nc.vector.tensor_copy(attE[:, n, :], aps[0:64, 0:P])
                # partition-shift odd heads up: attT[(n%2)*64+d, n//2, q]
                attT = opool.tile([P, 4, P], dt.float16, tag="attT")
                if debug and qb == 0:
                    nc.sync.dma_start(dPT[:], PT[:])
                    nc.sync.dma_start(dattE[:], attE[:])
                nc.sync.dma_start(attT[0:64, :, :], attE[:, 0:8:2, :])
                nc.sync.dma_start(attT[64:128, :, :], attE[:, 1:8:2, :])

                # Wout projection: out[q, :] = att @ Wout + bout
                pso = psc.tile([P, 512], dt.float32, tag="psx")
                for c in range(4):
                    nc.tensor.matmul(pso[:], attT[:, c, :], wo_sb[:, c, :],
                                     start=(c == 0), stop=(c == 3))
                outs = opool.tile([P, D], dt.float32, tag="outs")
                nc.vector.tensor_tensor(outs[:], pso[:], bot_sb[:], AL.add)
                nc.sync.dma_start(out_d[qsl, :], outs[:])

    nc.compile()
    return nc


def _host_prep(center_context, right_context, lengths, summary, memory,
               attention_mask, Wq, bq, Wkv, bkv, Wout, bout,
               pre_softmax_weight, post_softmax_weight):
    """Build the 8 per-core input maps (all numpy)."""
    f16 = np.float16
    q_in = np.concatenate([right_context, center_context, summary], 0)   # (Q,B,D)
    kv_in = np.concatenate([memory, right_context, center_context], 0)   # (K,B,D)

    # padding mask
    rcb = Q - int(np.max(lengths)) - S
    key_lengths = lengths.astype(np.int64) + M + rcb                     # (B,)
    kidx = np.arange(KP)
    am = np.zeros((Q, KP), dtype=bool)
    am[:, :K] = attention_mask
    am[:, K:] = True

    pre_bd = np.zeros((P, P), dtype=f16)
    post_bd = np.zeros((P, P), dtype=f16)
    for g in range(16):
        pre_bd[g * 8:(g + 1) * 8, g * 8:(g + 1) * 8] = pre_softmax_weight.astype(f16)
        post_bd[g * 8:(g + 1) * 8, g * 8:(g + 1) * 8] = post_softmax_weight.astype(f16)
    rep = np.zeros((P, 8 * P), dtype=f16)
    for qq in range(P):
        t, qlo = divmod(qq, 16)
        rep[qq, t * P + qlo * 8: t * P + qlo * 8 + 8] = 1.0

    wq16 = Wq.astype(f16)
    wk16 = Wkv[:, :D].astype(f16)
    wv16 = Wkv[:, D:].astype(f16)
    wo16 = Wout.astype(f16)
    bq16 = (bq * SCALE).astype(np.float32)
    bk16 = bkv[:D].astype(np.float32)
    bvt = np.broadcast_to(bkv[D:].astype(f16), (P, D)).copy()
    bot = np.broadcast_to(bout.astype(np.float32), (P, D)).copy()

    in_maps = []
    for core in range(8):
        b = core // 4
        q0 = (core % 4) * QSH
        qT = np.zeros((D, QP), dtype=f16)
        qT[:, :QSH] = q_in[q0:q0 + QSH, b, :].T.astype(f16)
        kvT = np.zeros((D, KP), dtype=f16)
        kvT[:, :K] = kv_in[:, b, :].T.astype(f16)
        mr = np.zeros((QP, KP), dtype=f16)
        bmask = am | (kidx[None, :] >= key_lengths[b])
        mr[:QSH] = np.where(bmask[q0:q0 + QSH], MASK_VAL, 0.0).astype(f16)
        in_maps.append({
            "qinT": qT, "kvinT": kvT, "wq": wq16, "wk": wk16, "wv": wv16,
            "wo": wo16, "bq": bq16, "bk": bk16, "bvt": bvt, "bot": bot,
            "maskr": mr, "prebd": pre_bd, "postbd": post_bd, "rep": rep,
        })
    return in_maps


def kernel(**inputs):
    import os
    os.environ.setdefault("BASS_NEVER_TRACE", "1")
    from concourse.bass_utils import run_bass_kernel_spmd

    inputs = {k: np.asarray(v) for k, v in inputs.items()}
    in_maps = _host_prep(**inputs)

    if "nc" not in _BUILT:
        _BUILT["nc"] = _build_program()
    nc = _BUILT["nc"]

    res = run_bass_kernel_spmd(nc, in_maps, list(range(8)))
    _BUILT["last_result"] = res

    out_full = np.zeros((Q, B, D), dtype=np.float32)
    for core in range(8):
        b = core // 4
        q0 = (core % 4) * QSH
        out_full[q0:q0 + QSH, b, :] = res.results[core]["out"][:QSH]

    output = out_full[:Q - S]
    out_mem = np.clip(out_full[Q - S:Q - 1], -10.0, 10.0)
    return output, out_mem
